# revision 6
# baseline (speedup 1.0000x reference)
"""Trainium2 Bass kernel for nn_Connect_Cls (GNN edge-pair classifier).

Math refactor: for pairs (i, j),
    h[e] = concat(x[i], x[j]) @ W1 + b1 = (x @ W1_top)[i] + (x @ W1_bot)[j] + b1
so we precompute per-node tables A = x @ W1[:512], B = x @ W1[512:] (sharded
over nodes, AllGathered), then each edge is a gather + add.  b1 cancels out of
the BatchNorm entirely (it shifts h and mu equally), so it is never used.

Per core (8 cores, data-parallel over the 131072 edge pairs):
  phase 1: compute a 1024-node shard of the combined [8192, 2048] bf16 AB
           table on the PE (bf16), AllGather the full table.
  pass 1:  dma_gather (transposed: features on partitions) A[i] and B[j] rows,
           h = A + B on DVE, bn_stats for batch-norm statistics, spill h tiles
           to a DRAM scratch (bf16).
  stats:   bn_aggr -> per-core sum/sumsq, AllReduce, then per-feature
           scale = gamma*rsqrt(var+eps), shift = beta - mu*scale.
  pass 2:  reload h tiles, fused relu(scale*h + shift) on ACT (per-partition
           scale/bias), out = relu_h @ W2 on PE (contraction over features =
           partitions), + b2, write transposed output [2, E_core].
"""

import numpy as np

import concourse.bacc as bacc
import concourse.bass as bass
import concourse.mybir as mybir
import concourse.tile as tile
from concourse.bass_utils import run_bass_kernel_spmd
from concourse.library_config import mlp

f32 = mybir.dt.float32
bf16 = mybir.dt.bfloat16
i16 = mybir.dt.int16
OP = mybir.AluOpType
AF = mybir.ActivationFunctionType

N_NODES = 8192
F_IN = 512
F_MID = 1024
NCLS = 2
E = 65536
NCORES = 8
E_CORE = 2 * E // NCORES       # 16384 edges per core
NODES_CORE = N_NODES // NCORES  # 1024 nodes per core in phase 1
FC = F_MID // 128               # 8 feature chunks of 128
KC_IN = F_IN // 128             # 4 input-feature chunks
GE = 512                        # edges per gather tile
NT = E_CORE // GE               # 32 tiles
BN_EPS = 1e-5


def build_program():
    nc = bacc.Bacc("TRN2", target_bir_lowering=False, debug=False,
                   num_devices=NCORES)

    inp = nc.dram_tensor("inp_shard", [NODES_CORE, F_IN], f32, kind="ExternalInput")
    w1 = nc.dram_tensor("w1", [2 * F_IN, F_MID], f32, kind="ExternalInput")
    w2 = nc.dram_tensor("w2", [F_MID, NCLS], f32, kind="ExternalInput")
    gamma = nc.dram_tensor("gamma", [F_MID], f32, kind="ExternalInput")
    beta = nc.dram_tensor("beta", [F_MID], f32, kind="ExternalInput")
    b2 = nc.dram_tensor("b2", [NCLS], f32, kind="ExternalInput")
    idx_src = nc.dram_tensor("idx_src", [128, E_CORE // 16], i16, kind="ExternalInput")
    idx_dst = nc.dram_tensor("idx_dst", [128, E_CORE // 16], i16, kind="ExternalInput")
    outT = nc.dram_tensor("outT", [NCLS, E_CORE], f32, kind="ExternalOutput")

    groups = [list(range(NCORES))]

    with tile.TileContext(nc) as tc:
        with (
            tc.tile_pool(name="const", bufs=1) as cs,
            tc.tile_pool(name="sb", bufs=1) as sb,
            tc.tile_pool(name="psum", bufs=2, space="PSUM") as pp,
            tc.tile_pool(name="dram", bufs=1, space="DRAM") as dram,
        ):
            nc.gpsimd.load_library(mlp)

            # ---------------- phase 1: node tables ----------------
            from concourse.masks import make_identity
            ident = cs.tile([128, 128], bf16)
            make_identity(nc, ident[:])

            w1_sb = cs.tile([128, 2 * KC_IN, F_MID], bf16)  # W1 rows chunked
            for kc in range(2 * KC_IN):
                nc.gpsimd.dma_start(out=w1_sb[:, kc, :],
                                    in_=w1[kc * 128:(kc + 1) * 128, :])

            # transpose input shard: inT[:, kk, n] = x[n, kk*128 + p]
            inT = cs.tile([128, KC_IN, NODES_CORE], bf16)
            for t in range(NODES_CORE // 128):
                xin = sb.tile([128, F_IN], bf16, tag="xin", bufs=2)
                nc.gpsimd.dma_start(out=xin[:], in_=inp[t * 128:(t + 1) * 128, :])
                for kk in range(KC_IN):
                    tps = pp.tile([128, 128], bf16, tag="tps", bufs=2)
                    nc.tensor.transpose(out=tps[:], in_=xin[:, kk * 128:(kk + 1) * 128],
                                        identity=ident[:])
                    nc.vector.tensor_copy(out=inT[:, kk, t * 128:(t + 1) * 128],
                                          in_=tps[:])

            ab_shard = dram.tile([NODES_CORE, 2 * F_MID], bf16)
            for t in range(NODES_CORE // 128):
                for half in range(2):           # A then B
                    for ofc in range(2):        # 512-wide output chunks
                        mmps = pp.tile([128, 512], f32, tag="mmps", bufs=2)
                        for kk in range(KC_IN):
                            nc.tensor.matmul(
                                out=mmps[:],
                                lhsT=inT[:, kk, t * 128:(t + 1) * 128],
                                rhs=w1_sb[:, half * KC_IN + kk,
                                          ofc * 512:(ofc + 1) * 512],
                                start=(kk == 0), stop=(kk == KC_IN - 1),
                            )
                        absb = sb.tile([128, 512], bf16, tag="absb", bufs=3)
                        nc.any.tensor_copy(out=absb[:], in_=mmps[:])
                        nc.sync.dma_start(
                            out=ab_shard[t * 128:(t + 1) * 128,
                                         half * F_MID + ofc * 512:
                                         half * F_MID + (ofc + 1) * 512],
                            in_=absb[:])

            ab_full = dram.tile([N_NODES, 2 * F_MID], bf16, addr_space="Shared")
            nc.gpsimd.collective_compute(
                "AllGather", OP.bypass, replica_groups=groups,
                ins=[ab_shard.opt()], outs=[ab_full.opt()])

            # ---------------- pass 1: gather + h + stats ----------------
            idxs = cs.tile([128, 2, E_CORE // 16], i16)
            nc.sync.dma_start(out=idxs[:, 0, :], in_=idx_src[:])
            nc.sync.dma_start(out=idxs[:, 1, :], in_=idx_dst[:])

            h_scr = dram.tile([NT, 128, FC, GE], bf16)
            stats = cs.tile([128, FC, NT, 6], f32)
            for g in range(NT):
                ag = sb.tile([128, FC, GE], bf16, tag="ag", bufs=2)
                bg = sb.tile([128, FC, GE], bf16, tag="bg", bufs=2)
                isl = slice(g * (GE // 16), (g + 1) * (GE // 16))
                nc.gpsimd.dma_gather(
                    ag[:], ab_full[:, 0:F_MID], idxs[:, 0, isl],
                    GE, GE, F_MID, elem_step=2 * F_MID, transpose=True)
                nc.gpsimd.dma_gather(
                    bg[:], ab_full[:, F_MID:2 * F_MID], idxs[:, 1, isl],
                    GE, GE, F_MID, elem_step=2 * F_MID, transpose=True)
                h = sb.tile([128, FC, GE], bf16, tag="h", bufs=3)
                nc.vector.tensor_tensor(out=h[:], in0=ag[:], in1=bg[:], op=OP.add)
                for c in range(FC):
                    nc.vector.bn_stats(out=stats[:, c, g, :], in_=h[:, c, :])
                nc.sync.dma_start(out=h_scr[g], in_=h[:])

            # ---------------- stats: aggregate + AllReduce ----------------
            mv = cs.tile([128, FC, 2], f32)
            for c in range(FC):
                nc.vector.bn_aggr(out=mv[:, c, :], in_=stats[:, c, :, :])
            ar_sb = cs.tile([128, 2 * FC], f32)
            msq = cs.tile([128, FC], f32)
            nc.vector.tensor_scalar_mul(out=ar_sb[:, 0:FC], in0=mv[:, :, 0],
                                        scalar1=float(E_CORE))
            nc.vector.tensor_tensor(out=msq[:], in0=mv[:, :, 0], in1=mv[:, :, 0],
                                    op=OP.mult)
            nc.vector.tensor_tensor(out=msq[:], in0=msq[:], in1=mv[:, :, 1],
                                    op=OP.add)
            nc.vector.tensor_scalar_mul(out=ar_sb[:, FC:2 * FC], in0=msq[:],
                                        scalar1=float(E_CORE))
            ar_in = dram.tile([128, 2 * FC], f32)
            ar_out = dram.tile([128, 2 * FC], f32, addr_space="Shared")
            nc.sync.dma_start(out=ar_in[:], in_=ar_sb[:])
            nc.gpsimd.collective_compute(
                "AllReduce", OP.add, replica_groups=groups,
                ins=[ar_in.opt()], outs=[ar_out.opt()])
            gsum = cs.tile([128, 2 * FC], f32)
            nc.sync.dma_start(out=gsum[:], in_=ar_out[:])

            mu = cs.tile([128, FC], f32)
            var = cs.tile([128, FC], f32)
            inv_n = 1.0 / (2.0 * E)
            nc.vector.tensor_scalar_mul(out=mu[:], in0=gsum[:, 0:FC], scalar1=inv_n)
            nc.vector.tensor_scalar_mul(out=var[:], in0=gsum[:, FC:2 * FC],
                                        scalar1=inv_n)
            musq = cs.tile([128, FC], f32)
            nc.vector.tensor_tensor(out=musq[:], in0=mu[:], in1=mu[:], op=OP.mult)
            nc.vector.tensor_tensor(out=var[:], in0=var[:], in1=musq[:],
                                    op=OP.subtract)
            eps_t = cs.tile([128, 1], f32)
            nc.gpsimd.memset(eps_t[:], BN_EPS)
            std = cs.tile([128, FC], f32)
            nc.scalar.activation(out=std[:], in_=var[:], func=AF.Sqrt,
                                 bias=eps_t[:, 0:1])
            rstd = cs.tile([128, FC], f32)
            nc.vector.reciprocal(out=rstd[:], in_=std[:])

            gam = cs.tile([128, FC], f32)
            bet = cs.tile([128, FC], f32)
            nc.sync.dma_start(out=gam[:], in_=gamma[:].rearrange("(c p) -> p c", p=128))
            nc.sync.dma_start(out=bet[:], in_=beta[:].rearrange("(c p) -> p c", p=128))
            scale = cs.tile([128, FC], f32)
            shift = cs.tile([128, FC], f32)
            nc.vector.tensor_tensor(out=scale[:], in0=gam[:], in1=rstd[:], op=OP.mult)
            nc.vector.tensor_tensor(out=shift[:], in0=mu[:], in1=scale[:], op=OP.mult)
            nc.vector.tensor_tensor(out=shift[:], in0=bet[:], in1=shift[:],
                                    op=OP.subtract)

            # ---------------- pass 2: relu-affine + W2 ----------------
            w2_sb = cs.tile([128, FC, NCLS], bf16)
            for c in range(FC):
                nc.gpsimd.dma_start(out=w2_sb[:, c, :],
                                    in_=w2[c * 128:(c + 1) * 128, :])
            b2_sb = cs.tile([NCLS, 1], f32)
            nc.sync.dma_start(out=b2_sb[:], in_=b2[:, None])

            for g in range(NT):
                hh = sb.tile([128, FC, GE], bf16, tag="hh", bufs=3)
                nc.sync.dma_start(out=hh[:], in_=h_scr[g])
                ops = pp.tile([NCLS, GE], f32, tag="ops", bufs=2)
                for c in range(FC):
                    hr = sb.tile([128, GE], bf16, tag="hr", bufs=3)
                    nc.scalar.activation(out=hr[:], in_=hh[:, c, :], func=AF.Relu,
                                         scale=scale[:, c:c + 1],
                                         bias=shift[:, c:c + 1])
                    nc.tensor.matmul(out=ops[:], lhsT=w2_sb[:, c, :], rhs=hr[:],
                                     start=(c == 0), stop=(c == FC - 1))
                ob = sb.tile([NCLS, GE], f32, tag="ob", bufs=3)
                nc.vector.tensor_scalar(out=ob[:], in0=ops[:],
                                        scalar1=b2_sb[:, 0:1], scalar2=None,
                                        op0=OP.add)
                nc.sync.dma_start(out=outT[:, g * GE:(g + 1) * GE], in_=ob[:])

    nc.compile()
    return nc


_NC = None


def _get_program():
    global _NC
    if _NC is None:
        _NC = build_program()
    return _NC


def _wrap_idx(col):
    """[E_CORE] int -> [128, E_CORE//16] int16 in dma_gather's wrapped layout."""
    w = col.astype(np.int16).reshape(-1, 16).T          # [16, E_CORE//16]
    return np.ascontiguousarray(np.tile(w, (8, 1)))     # replicate to 128 parts


def make_in_maps(input, conn_idx, disconn_idx, W1, gamma, beta, W2, b2):
    input = np.ascontiguousarray(np.asarray(input, dtype=np.float32))
    W1 = np.ascontiguousarray(np.asarray(W1, dtype=np.float32))
    W2 = np.ascontiguousarray(np.asarray(W2, dtype=np.float32))
    gamma = np.ascontiguousarray(np.asarray(gamma, dtype=np.float32))
    beta = np.ascontiguousarray(np.asarray(beta, dtype=np.float32))
    b2 = np.ascontiguousarray(np.asarray(b2, dtype=np.float32))
    conn_idx = np.asarray(conn_idx)
    disconn_idx = np.asarray(disconn_idx)

    in_maps = []
    ec2 = E_CORE // 2  # edges per core from each of conn/disconn
    for c in range(NCORES):
        pc = np.concatenate(
            [conn_idx[c * ec2:(c + 1) * ec2], disconn_idx[c * ec2:(c + 1) * ec2]],
            axis=0)  # [E_CORE, 2]
        in_maps.append({
            "inp_shard": np.ascontiguousarray(
                input[c * NODES_CORE:(c + 1) * NODES_CORE]),
            "w1": W1, "w2": W2, "gamma": gamma, "beta": beta, "b2": b2,
            "idx_src": _wrap_idx(pc[:, 0]),
            "idx_dst": _wrap_idx(pc[:, 1]),
        })
    return in_maps


def assemble_output(results):
    out = np.empty((2 * E, NCLS), dtype=np.float32)
    ec2 = E_CORE // 2
    for c in range(NCORES):
        r = results[c]["outT"]  # [NCLS, E_CORE]
        out[c * ec2:(c + 1) * ec2] = r[:, 0:ec2].T
        out[E + c * ec2:E + (c + 1) * ec2] = r[:, ec2:].T
    return out


def run(inputs, trace=False):
    nc = _get_program()
    in_maps = make_in_maps(
        inputs["input"], inputs["conn_idx"], inputs["disconn_idx"],
        inputs["W1"], inputs["gamma"], inputs["beta"], inputs["W2"],
        inputs["b2"])
    res = run_bass_kernel_spmd(nc, in_maps, list(range(NCORES)), trace=trace)
    return assemble_output(res.results), res


def kernel(**inputs):
    out, _ = run(inputs, trace=False)
    return out


def bench(inputs, iters=10):
    """Steady-state timing of the compiled NEFF via the PJRT path.

    Inputs are device-put once; each iteration supplies fresh (tiny) zeroed
    output buffers since those are donated. Returns per-iteration seconds.
    """
    import time
    import jax
    from jax.sharding import Mesh, PartitionSpec
    from jax.experimental.shard_map import shard_map
    from concourse import bass2jax
    from concourse import mybir as mb

    nc = _get_program()
    in_maps = make_in_maps(
        inputs["input"], inputs["conn_idx"], inputs["disconn_idx"],
        inputs["W1"], inputs["gamma"], inputs["beta"], inputs["W2"],
        inputs["b2"])

    bass2jax.install_neuronx_cc_hook()
    partition_name = nc.partition_id_tensor.name if nc.partition_id_tensor else None
    in_names, out_names, out_avals, zero_outs = [], [], [], []
    for alloc in nc.m.functions[0].allocations:
        if not isinstance(alloc, mb.MemoryLocationSet):
            continue
        kind = alloc.kind
        if not alloc.memorylocations:
            continue
        name = alloc.memorylocations[0].name
        if kind == "ExternalInput":
            if name != partition_name:
                in_names.append(name)
        elif kind == "ExternalOutput":
            shape = tuple(alloc.tensor_shape)
            dtype = mybir.dt.np(alloc.dtype)
            out_names.append(name)
            out_avals.append(jax.core.ShapedArray(shape, dtype))
            zero_outs.append(np.zeros(shape, dtype))
    n_params = len(in_names)
    n_outs = len(out_avals)
    in_names_full = list(in_names) + list(out_names)
    if partition_name is not None:
        in_names_full.append(partition_name)

    from concourse.bass2jax import _bass_exec_p, partition_id_tensor

    def _body(*args):
        operands = list(args)
        if partition_name is not None:
            operands.append(partition_id_tensor())
        outs = _bass_exec_p.bind(
            *operands,
            out_avals=tuple(out_avals),
            in_names=tuple(in_names_full),
            out_names=tuple(out_names),
            lowering_input_output_aliases=(),
            sim_require_finite=True,
            sim_require_nnan=True,
            nc=nc,
        )
        return tuple(outs)

    devices = jax.devices()[:NCORES]
    mesh = Mesh(np.asarray(devices), ("core",))
    in_specs = (PartitionSpec("core"),) * (n_params + n_outs)
    out_specs = (PartitionSpec("core"),) * len(out_names)
    donate = tuple(range(n_params, n_params + n_outs))
    fn = jax.jit(
        shard_map(_body, mesh=mesh, in_specs=in_specs, out_specs=out_specs,
                  check_rep=False),
        donate_argnums=donate, keep_unused=True)

    concat_in = [
        np.concatenate([np.asarray(in_maps[c][nm]) for c in range(NCORES)], axis=0)
        for nm in in_names
    ]
    concat_in = [jax.device_put(a) for a in concat_in]
    for a in concat_in:
        a.block_until_ready()

    def fresh_zeros():
        return [np.zeros((NCORES * z.shape[0], *z.shape[1:]), z.dtype)
                for z in zero_outs]

    # warmup
    outs = fn(*concat_in, *fresh_zeros())
    jax.block_until_ready(outs)
    times = []
    for _ in range(iters):
        zs = fresh_zeros()
        t0 = time.perf_counter()
        outs = fn(*concat_in, *zs)
        jax.block_until_ready(outs)
        times.append(time.perf_counter() - t0)
    return times
